# revision 1
# baseline (speedup 1.0000x reference)
"""Trainium2 Bass kernel for nn_CNNQNetwork (dense_cnn).

Same math as the reference: 7 small VALID convs on a fixed 4x4x16 input are
one structured 256->3712 linear map (the conv stage), followed by the
3-layer MLP 3712->512->128->4.  Pure data parallel over 8 NeuronCores
(4096 samples each), weights replicated.  All matmuls are f32r (fp32
streamed at 1 col/cycle for N>=256), fp32 PSUM accumulation.

Key structure (vs a phase-per-layer schedule):

- Chunk-interleaved software pipeline: L1 chunk j (one 128x128 matmul from
  a permuted x layout) is drained (relu+bias, alternating ACT/DVE) and
  immediately contracted into the four persistent L2 PSUM accumulators.
  The L1 producer runs 2 chunks ahead of the L2 consumer so every drain's
  cross-engine round trip is covered by PE work; no phase is drain-bound
  and no buffer is ever WAR-blocked at a tile boundary.

- L2 is emitted as same-accumulator runs of up to 4 chunks with the next
  group's L1 matmuls interleaved one per run.

- Each L1 chunk covers 2 conv output positions whose input support fits a
  single 128-row half of one of three shipped x layouts (A: row-major
  positions, B: column-major, E: a gathered layout holding the one 2x2 conv
  chunk that straddles A and B halves) -> exactly 29 L1 matmuls per tile.

- The tile-t tail is split across the t+1 boundary: L3 + the a3 drain
  run at the boundary (covering the L2-accumulator WAR round trip) and
  L4 + the output DMA one chunk-group later, so the a3 drain's round
  trip is covered by ~16 L2 matmuls.

- Weights load once on the sync DGE queue in consumption-need order
  (grouped DMAs); x streams per-tile on the gpsimd/SWDGE queue, prefetched
  one tile ahead.  In the repeat-loop steady state there is no weight
  traffic at all.

Per-tile PE work: 29 (L1) + 116 (L2) + 5 (L3+L4) = 150 matmuls x 512
columns at 1 col/cycle; 8 tiles per core per pass.
"""

import numpy as np
import ml_dtypes

try:
    import jax as _jax

    _jax.config.update("jax_compilation_cache_dir", "/tmp/jax_cache")
    _jax.config.update("jax_persistent_cache_min_compile_time_secs", 2)
    _jax.config.update("jax_persistent_cache_min_entry_size_bytes", 0)
except Exception:
    pass

import concourse.bass as bass
import concourse.bacc as bacc
import concourse.mybir as mybir
import concourse.tile as tile
from concourse.bass import ts
from concourse.bass_utils import run_bass_kernel_spmd

N_CORES = 8
B = 32768
B_LOC = B // N_CORES  # 4096
NB = 512
BT = B_LOC // NB  # 8
P = 128
F_IN = 256
K1 = F_IN // P  # 2
H1 = 3712
M1 = H1 // P  # 29
H2 = 512
M2 = H2 // P  # 4
H3 = 128
NA = 4

F32 = mybir.dt.float32
F32R = mybir.dt.float32r
BF16 = mybir.dt.bfloat16
BF_NP = ml_dtypes.bfloat16

KERNELS = [(1, 2), (2, 1), (1, 3), (3, 1), (1, 4), (4, 1), (2, 2)]
OFFSETS = np.cumsum([0] + [64 * (5 - kh) * (5 - kw) for kh, kw in KERNELS])

# chunk plan: 29 chunks x (conv idx, [2 output positions or None]).
# Layout is chosen per chunk: "A" (row-major positions), "B" (col-major), or
# "E" (a gathered extra layout holding the straddling chunks' supports).
_L1_PLAN = [
    (0, "A", [(0, 0), (0, 1)]), (0, "A", [(0, 2), (1, 0)]),
    (0, "A", [(1, 1), (1, 2)]), (0, "A", [(2, 0), (2, 1)]),
    (0, "A", [(2, 2), (3, 0)]), (0, "A", [(3, 1), (3, 2)]),
    (1, "B", [(0, 0), (1, 0)]), (1, "B", [(2, 0), (0, 1)]),
    (1, "B", [(1, 1), (2, 1)]), (1, "B", [(0, 2), (1, 2)]),
    (1, "B", [(2, 2), (0, 3)]), (1, "B", [(1, 3), (2, 3)]),
    (2, "A", [(0, 0), (0, 1)]), (2, "A", [(1, 0), (1, 1)]),
    (2, "A", [(2, 0), (2, 1)]), (2, "A", [(3, 0), (3, 1)]),
    (3, "B", [(0, 0), (1, 0)]), (3, "B", [(0, 1), (1, 1)]),
    (3, "B", [(0, 2), (1, 2)]), (3, "B", [(0, 3), (1, 3)]),
    (4, "A", [(0, 0), (1, 0)]), (4, "A", [(2, 0), (3, 0)]),
    (5, "B", [(0, 0), (0, 1)]), (5, "B", [(0, 2), (0, 3)]),
    (6, "A", [(0, 0), (0, 1)]), (6, "B", [(0, 2), (1, 2)]),
    (6, "B", [(1, 0), (2, 0)]), (6, "A", [(2, 1), (2, 2)]),
    (6, "A", [(1, 1), None]),
]
assert len(_L1_PLAN) == M1


def _pos(lay, h, w):
    return h * 4 + w if lay == "A" else w * 4 + h


def _chunk_support(ci, grp):
    """Set of (h, w) input positions this chunk's outputs read."""
    kh, kw = KERNELS[ci]
    sup = set()
    for o in grp:
        if o is None:
            continue
        pi, pj = o
        for dh in range(kh):
            for dw in range(kw):
                sup.add((pi + dh, pj + dw))
    return sup


def _build_plan():
    """Resolve each chunk to (layout, half, rows) where rows maps each support
    position to a row block.  Chunks that fit a single 128-row half of layout
    A or B use it; the rest go to the gathered extra layout E (single half).
    Returns the per-chunk source list and the E layout's position list.
    """
    srcs = []
    e_positions = []  # position order within layout E
    for ci, lay_hint, grp in _L1_PLAN:
        sup = _chunk_support(ci, grp)
        placed = None
        for lay in ("A", "B"):
            halves = {_pos(lay, h, w) // 8 for h, w in sup}
            if len(halves) == 1:
                placed = (lay, halves.pop())
                break
        if placed is None:
            # gathered layout: append this chunk's support positions
            for p in sorted(sup):
                if p not in e_positions:
                    e_positions.append(p)
            placed = ("E", 0)
        srcs.append(placed)
    assert len(e_positions) <= 8, e_positions
    return srcs, e_positions


_SRCS, _E_POSITIONS = _build_plan()
N_XE = 1 if _E_POSITIONS else 0


def _row_of(lay, h, w):
    """Row (0..127) of position (h,w), channel 0, within its half of layout."""
    if lay == "E":
        return _E_POSITIONS.index((h, w)) * 16
    p = _pos(lay, h, w)
    return (p - (p // 8) * 8) * 16


def _build_l1_weights(ws, bs):
    """Wc [M1,128,128] bf16 (K-row, M-col), bias bc [M1,128] f32, perm."""
    Wc = np.zeros((M1, P, P), np.float32)
    bch = np.zeros((M1, P), np.float32)
    perm = np.full(H1, -1, np.int64)
    for j, ((ci, _, grp), (lay, half)) in enumerate(zip(_L1_PLAN, _SRCS)):
        kh, kw = KERNELS[ci]
        oh, ow = 5 - kh, 5 - kw
        for sl, o in enumerate(grp):
            if o is None:
                continue
            pi, pj = o
            cols = slice(sl * 64, sl * 64 + 64)
            bch[j, cols] = bs[ci]
            perm[j * P + sl * 64 : j * P + sl * 64 + 64] = (
                OFFSETS[ci] + np.arange(64) * oh * ow + pi * ow + pj
            )
            for dh in range(kh):
                for dw in range(kw):
                    row = _row_of(lay, pi + dh, pj + dw)
                    Wc[j, row : row + 16, cols] = ws[ci][:, :, dh, dw].T
    return Wc, bch, perm


def _x_row_perm():
    """Per layout: new row r -> original flat-x feature index (c*16+h*4+w)."""
    out = {}
    for lay in ("A", "B"):
        a = np.empty(F_IN, np.int64)
        for r in range(F_IN):
            pos, c = r // 16, r % 16
            if lay == "A":
                h, w = pos // 4, pos % 4
            else:
                w, h = pos // 4, pos % 4
            a[r] = c * 16 + h * 4 + w
        out[lay] = a
    e = np.zeros(P, np.int64)
    for i, (h, w) in enumerate(_E_POSITIONS):
        for c in range(16):
            e[i * 16 + c] = c * 16 + h * 4 + w
    out["E"] = e
    return out


_X_PERMS = _x_row_perm()

_PROGRAM_CACHE = {}


# weight-DMA schedule: (tensor, lo, hi) in consumption-need order — small
# fw0 groups first (chunk j's L2 runs at ~3 + 1.07*j us), larger batches
# later to amortize the ~625ns HWDGE fixed cost per DMA (the HWDGE
# descriptor generator is a single shared unit).
_W_SCHED = [
    ("wc", 0, 1), ("cst", 0, 0), ("fw0", 0, 1), ("wc", 1, 4), ("fw0", 1, 2),
    ("fw0", 2, 3), ("wc", 4, 8), ("fw0", 3, 5), ("wc", 8, 16), ("fw0", 5, 8),
    ("wc", 16, M1), ("fw0", 8, 11), ("fw0", 11, 15), ("fw0", 15, 20),
    ("fw0", 20, M1), ("fw1", 0, 0), ("fw2", 0, 0),
]
# All weight DMAs run once on the sync queue (outside the repeat loop):
# pass 1 absorbs a few us of weight-arrival stalls, steady-state passes
# have no weight traffic at all.
NCONST = M1 + M2 + 2  # bc | fb0 | fb1 | fb2


def _build_program(repeat=1):
    nc = bacc.Bacc(None, target_bir_lowering=False)
    # x layouts are partition-major so a whole tile is one DMA
    xa_d = nc.declare_dram_parameter("xa", [P, K1, B_LOC], F32R, isOutput=False)
    xb_d = nc.declare_dram_parameter("xb", [P, K1, B_LOC], F32R, isOutput=False)
    xe_d = nc.declare_dram_parameter("xe", [P, B_LOC], F32R, isOutput=False)
    wc_d = nc.declare_dram_parameter("wc", [P, M1, P], F32R, isOutput=False)
    fw0_d = nc.declare_dram_parameter("fw0t", [P, M1, H2], F32R, isOutput=False)
    fw1_d = nc.declare_dram_parameter("fw1t", [P, M2, H3], F32R, isOutput=False)
    fw2_d = nc.declare_dram_parameter("fw2t", [P, NA], F32R, isOutput=False)
    cst_d = nc.declare_dram_parameter("cst", [P, NCONST], F32, isOutput=False)
    out_d = nc.declare_dram_parameter("out", [NA, B_LOC], F32, isOutput=True)

    RELU = mybir.ActivationFunctionType.Relu
    ADD = mybir.AluOpType.add
    MAX = mybir.AluOpType.max

    with tile.TileContext(nc) as tc:
        with (
            tc.tile_pool(name="wpool", bufs=1) as wpool,
            tc.tile_pool(name="xpool", bufs=2) as xpool,
            tc.tile_pool(name="a1pool", bufs=1) as a1pool,
            tc.tile_pool(name="a2pool", bufs=2) as a2pool,
            tc.tile_pool(name="a3pool", bufs=2) as a3pool,
            tc.tile_pool(name="opool", bufs=2) as opool,
            tc.tile_pool(name="ps1pool", bufs=3, space="PSUM") as ps1pool,
            tc.tile_pool(name="ps2pool", bufs=1, space="PSUM") as ps2pool,
            tc.tile_pool(name="pstail", bufs=1, space="PSUM") as pstail,
        ):
            # replicated weights, loaded once, resident for the whole pass,
            # in grouped DMAs (deps are per-slice, so chunk j's compute waits
            # only for the group containing j).  Tile 0's layout-A x block
            # goes first on the same queue: it gates the very first matmul
            # and the HWDGE path is ~1us faster than SWDGE.
            wc_all = wpool.tile([P, M1, P], F32R)
            fw0_all = wpool.tile([P, M1, H2], F32R)
            cst = wpool.tile([P, NCONST], F32)
            fw1 = wpool.tile([P, M2, H3], F32R)
            fw2 = wpool.tile([P, NA], F32R)
            for which, g0, g1 in _W_SCHED:
                if which == "wc":
                    nc.sync.dma_start(wc_all[:, g0:g1, :], wc_d[:, g0:g1, :])
                elif which == "fw0":
                    nc.sync.dma_start(fw0_all[:, g0:g1, :], fw0_d[:, g0:g1, :])
                elif which == "cst":
                    nc.sync.dma_start(cst[:], cst_d[:])
                elif which == "fw1":
                    nc.sync.dma_start(fw1[:], fw1_d[:])
                else:
                    nc.sync.dma_start(fw2[:], fw2_d[:])
            bc = cst[:, 0:M1]
            fb0 = cst[:, M1 : M1 + M2]
            fb1 = cst[:, M1 + M2 : M1 + M2 + 1]
            fb2 = cst[0:NA, M1 + M2 + 1 : M1 + M2 + 2]

            def tail_a(tp, a2p):
                # L3 + a3 drain for tile tp; emitted at the NEXT tile
                # boundary so its inputs are long drained and its matmuls
                # cover the a2-accumulator WAR round trip.
                ps3 = pstail.tile([P, NB], F32, tag="pst", name="ps3")
                for k in range(M2):
                    nc.tensor.matmul(
                        ps3[:], fw1[:, k, :], a2p[:, k, :],
                        start=(k == 0), stop=(k == M2 - 1),
                    )
                a3 = a3pool.tile([P, NB], F32R, tag="a3")
                nc.scalar.activation(a3[:], ps3[:], RELU, bias=fb1)
                return a3

            def tail_b(tp, a3):
                # L4 + output, emitted one chunk-group after tail_a so the
                # a3 drain's round trip is covered by ~16 L2 matmuls
                ps4 = pstail.tile([NA, NB], F32, tag="pst", name="ps4")
                nc.tensor.matmul(ps4[:], fw2[:], a3[:], start=True, stop=True)
                ob = opool.tile([NA, NB], F32, tag="ob")
                nc.vector.tensor_scalar_add(ob[:], ps4[:], fb2)
                nc.sync.dma_start(out_d[:, ts(tp, NB)], ob[:])

            def body():
                NCH = BT * M1  # flat chunk stream across all batch tiles
                xts = {}  # tile t -> (xta, xtb, xte)
                a1s = {}
                a2s = {}
                ps2s = {}

                def stage_tile(t):
                    # x DMAs + per-tile SBUF/PSUM allocations for tile t
                    if t >= BT:
                        return
                    xta = xpool.tile([P, K1, NB], F32R, tag="xta", name="xta")
                    if t == 0:
                        for k in range(K1):
                            nc.gpsimd.dma_start(
                                xta[:, k, :], xa_d[:, k, ts(t, NB)]
                            )
                    else:
                        nc.gpsimd.dma_start(xta[:], xa_d[:, :, ts(t, NB)])
                    xtb = xpool.tile([P, K1, NB], F32R, tag="xtb", name="xtb")
                    nc.gpsimd.dma_start(xtb[:], xb_d[:, :, ts(t, NB)])
                    xte = None
                    if N_XE:
                        xte = xpool.tile([P, NB], F32R, tag="xte", name="xte")
                        nc.gpsimd.dma_start(xte[:], xe_d[:, ts(t, NB)])
                    xts[t] = (xta, xtb, xte)
                    a1s[t] = a1pool.tile([P, M1, NB], F32R, tag="a1", name="a1")

                def l1(g):
                    # L1 chunk g: one matmul + relu/bias drain into a1
                    if g >= NCH:
                        return
                    t, j = divmod(g, M1)
                    lay, half = _SRCS[j]
                    xta, xtb, xte = xts[t]
                    if lay == "A":
                        xt = xta[:, half, :]
                    elif lay == "B":
                        xt = xtb[:, half, :]
                    else:
                        xt = xte[:]
                    ps = ps1pool.tile([P, NB], F32, tag="ps1", name="ps1")
                    nc.tensor.matmul(ps[:], wc_all[:, j, :], xt, start=True,
                                     stop=True)
                    a1 = a1s[t]
                    if g % 2 == 0:
                        nc.scalar.activation(
                            a1[:, j, :], ps[:], RELU, bias=bc[:, j : j + 1]
                        )
                    else:
                        nc.vector.tensor_scalar(
                            a1[:, j, :], ps[:], bc[:, j : j + 1], 0.0,
                            ADD, MAX,
                        )

                # chunk groups: L2 is emitted as same-accumulator RUNS of
                # up to G chunks (PSUM-target switches between consecutive
                # matmuls cost real overlap on HW), with the next group's
                # L1 matmuls interleaved one per run.
                G = 4
                GROUPS = [(s, min(s + G, M1)) for s in range(0, M1, G)]
                NGRP = len(GROUPS)

                def group_start(gi):
                    # global chunk index of group gi's first chunk
                    if gi >= BT * NGRP:
                        return NCH
                    t2, gl2 = divmod(gi, NGRP)
                    return t2 * M1 + GROUPS[gl2][0]

                stage_tile(0)
                stage_tile(1)
                for g in range(GROUPS[0][1]):
                    l1(g)
                next_l1 = GROUPS[0][1]
                for gi in range(BT * NGRP):
                    t, gl = divmod(gi, NGRP)
                    j0, j1 = GROUPS[gl]
                    if gl == 0:
                        if t > 0:
                            pending = (t - 1, tail_a(t - 1, a2s.pop(t - 1)))
                        # L2 accumulators allocated at first use so the ring
                        # WAR dep lands on the previous tile's a2 drains
                        ps2s[t] = [
                            ps2pool.tile([P, NB], F32, tag=f"ps2_{m}",
                                         name=f"ps2_{m}")
                            for m in range(M2)
                        ]
                    if gl == 1 and t > 0:
                        tail_b(*pending)
                    a1 = a1s[t]
                    ps2 = ps2s[t]
                    final_group = j1 == M1

                    def drain_a2(a2, m):
                        if m % 2 == 0:
                            nc.vector.tensor_scalar(
                                a2[:, m, :], ps2[m][:],
                                fb0[:, m : m + 1], 0.0, ADD, MAX,
                            )
                        else:
                            nc.scalar.activation(
                                a2[:, m, :], ps2[m][:], RELU,
                                bias=fb0[:, m : m + 1],
                            )

                    if final_group:
                        a2 = a2s[t] = a2pool.tile(
                            [P, M2, NB], F32R, tag="a2", name="a2"
                        )
                    l1_stop = group_start(gi + 2)
                    for m in range(M2):
                        for j in range(j0, j1):
                            nc.tensor.matmul(
                                ps2[m][:],
                                fw0_all[:, j, ts(m, P)],
                                a1[:, j, :],
                                start=(j == 0),
                                stop=(j == M1 - 1),
                            )
                        if final_group:
                            # drain right after the stop matmul: frees the
                            # accumulator early and lets the tail's L3 chase
                            drain_a2(a2, m)
                        # next group's L1 chunks, one per run
                        if next_l1 < l1_stop:
                            l1(next_l1)
                            next_l1 += 1
                    while next_l1 < l1_stop:
                        l1(next_l1)
                        next_l1 += 1
                    if final_group:
                        del ps2s[t]
                        stage_tile(t + 2)
                tail_b(BT - 1, tail_a(BT - 1, a2s.pop(BT - 1)))

            if repeat == 1:
                body()
            else:
                with tc.For_i(0, repeat, 1):
                    body()

    nc.finalize()
    return nc


def pack_inputs(x, ws, bs, fw0, fb0, fw1, fb1, fw2, fb2):
    """Pack full-problem numpy inputs into the per-core DRAM in_maps."""
    x = np.asarray(x, np.float32).reshape(B, F_IN)
    ws = [np.asarray(w, np.float32) for w in ws]
    bs = [np.asarray(b, np.float32) for b in bs]
    fw0 = np.asarray(fw0, np.float32)
    fb0 = np.asarray(fb0, np.float32)
    fw1 = np.asarray(fw1, np.float32)
    fb1 = np.asarray(fb1, np.float32)
    fw2 = np.asarray(fw2, np.float32)
    fb2 = np.asarray(fb2, np.float32)

    Wc, bch, perm = _build_l1_weights(ws, bs)

    fw0_perm = np.zeros((H2, H1), np.float32)
    valid = perm >= 0
    fw0_perm[:, valid] = fw0[:, perm[valid]]

    # consts [P, NCONST]: bc | fb0 | fb1 | fb2 (fb2 in rows 0..NA-1)
    cst = np.zeros((P, NCONST), np.float32)
    cst[:, :M1] = bch.T
    cst[:, M1 : M1 + M2] = fb0.reshape(M2, P).T
    cst[:, M1 + M2] = fb1
    cst[:NA, M1 + M2 + 1] = fb2

    shared = {
        # partition-major weight layouts: [P(K), chunk, cols]
        "wc": np.ascontiguousarray(Wc.transpose(1, 0, 2)),
        "fw0t": np.ascontiguousarray(
            fw0_perm.T.reshape(M1, P, H2).transpose(1, 0, 2)
        ),
        "fw1t": np.ascontiguousarray(
            fw1.T.reshape(M2, P, H3).transpose(1, 0, 2)
        ),
        "fw2t": np.ascontiguousarray(fw2.T),
        "cst": cst,
    }
    xbf = x
    in_maps = []
    for i in range(N_CORES):
        shard_t = xbf[i * B_LOC : (i + 1) * B_LOC].T  # [256, B_LOC] view
        xa = np.ascontiguousarray(
            shard_t[_X_PERMS["A"]].reshape(K1, P, B_LOC).transpose(1, 0, 2)
        )
        xb = np.ascontiguousarray(
            shard_t[_X_PERMS["B"]].reshape(K1, P, B_LOC).transpose(1, 0, 2)
        )
        xe = np.ascontiguousarray(shard_t[_X_PERMS["E"]])
        in_maps.append({"xa": xa, "xb": xb, "xe": xe, **shared})
    return in_maps


def kernel(x, w0, b0, w1, b1, w2, b2, w3, b3, w4, b4, w5, b5, w6, b6,
           fw0, fb0, fw1, fb1, fw2, fb2):
    in_maps = pack_inputs(
        x, (w0, w1, w2, w3, w4, w5, w6), (b0, b1, b2, b3, b4, b5, b6),
        fw0, fb0, fw1, fb1, fw2, fb2,
    )
    if "nc" not in _PROGRAM_CACHE:
        _PROGRAM_CACHE["nc"] = _build_program()
    nc = _PROGRAM_CACHE["nc"]

    res = run_bass_kernel_spmd(nc, in_maps, list(range(N_CORES)))
    out = np.concatenate([r["out"] for r in res.results], axis=1)  # [4, B]
    return np.ascontiguousarray(out.T)



# revision 7
# speedup vs baseline: 1.8678x; 1.8678x over previous
"""Trainium2 Bass kernel for nn_CNNQNetwork (dense_cnn).

Same math as the reference: 7 small VALID convs on a fixed 4x4x16 input are
one structured 256->3712 linear map (the conv stage), followed by the
3-layer MLP 3712->512->128->4.  Pure data parallel over 8 NeuronCores
(4096 samples each), weights replicated.  All matmul operands are bf16
(1 col/cycle stream, same as f32r, but the weight-load path gets FWL so
back-to-back LDWEIGHTS hide under the 512-col streams), fp32 PSUM
accumulation; final output assembled in fp32.

Key structure (vs a phase-per-layer schedule):

- Chunk-interleaved software pipeline: L1 chunk j (one 128x128 matmul from
  a permuted x layout) is drained (relu+bias, alternating ACT/DVE) and
  immediately contracted into the four persistent L2 PSUM accumulators.
  The L1 producer runs 2 chunks ahead of the L2 consumer so every drain's
  cross-engine round trip is covered by PE work; no phase is drain-bound
  and no buffer is ever WAR-blocked at a tile boundary.

- L2 is emitted as same-accumulator runs of up to 4 chunks with the next
  group's L1 matmuls interleaved one per run.

- Each L1 chunk covers 2 conv output positions whose input support fits a
  single 128-row half of one of three shipped x layouts (A: row-major
  positions, B: column-major, E: a gathered layout holding the one 2x2 conv
  chunk that straddles A and B halves) -> exactly 29 L1 matmuls per tile.

- The tile-t tail is split across the t+1 boundary: L3 + the a3 drain
  run at the boundary (covering the L2-accumulator WAR round trip) and
  L4 + the output DMA one chunk-group later, so the a3 drain's round
  trip is covered by ~16 L2 matmuls.

- Weights load once on the sync DGE queue in consumption-need order
  (grouped DMAs); x streams per-tile on the gpsimd/SWDGE queue, prefetched
  one tile ahead.  In the repeat-loop steady state there is no weight
  traffic at all.

Per-tile PE work: 29 (L1) + 116 (L2) + 5 (L3+L4) = 150 matmuls x 512
columns at 1 col/cycle; 8 tiles per core per pass.
"""

import numpy as np
import ml_dtypes

try:
    import jax as _jax

    _jax.config.update("jax_compilation_cache_dir", "/tmp/jax_cache")
    _jax.config.update("jax_persistent_cache_min_compile_time_secs", 2)
    _jax.config.update("jax_persistent_cache_min_entry_size_bytes", 0)
except Exception:
    pass

import concourse.bass as bass
import concourse.bacc as bacc
import concourse.mybir as mybir
import concourse.tile as tile
from concourse.bass import ts
from concourse.bass_utils import run_bass_kernel_spmd

N_CORES = 8
B = 32768
B_LOC = B // N_CORES  # 4096
NB = 512
BT = B_LOC // NB  # 8
P = 128
F_IN = 256
K1 = F_IN // P  # 2
H1 = 3712
M1 = H1 // P  # 29
H2 = 512
M2 = H2 // P  # 4
H3 = 128
NA = 4

F32 = mybir.dt.float32
F32R = mybir.dt.float32r
BF16 = mybir.dt.bfloat16
BF_NP = ml_dtypes.bfloat16

KERNELS = [(1, 2), (2, 1), (1, 3), (3, 1), (1, 4), (4, 1), (2, 2)]
OFFSETS = np.cumsum([0] + [64 * (5 - kh) * (5 - kw) for kh, kw in KERNELS])

# chunk plan: 29 chunks x (conv idx, [2 output positions or None]).
# Layout is chosen per chunk: "A" (row-major positions), "B" (col-major), or
# "E" (a gathered extra layout holding the straddling chunks' supports).
_L1_PLAN = [
    (0, "A", [(0, 0), (0, 1)]), (0, "A", [(0, 2), (1, 0)]),
    (0, "A", [(1, 1), (1, 2)]), (0, "A", [(2, 0), (2, 1)]),
    (0, "A", [(2, 2), (3, 0)]), (0, "A", [(3, 1), (3, 2)]),
    (1, "B", [(0, 0), (1, 0)]), (1, "B", [(2, 0), (0, 1)]),
    (1, "B", [(1, 1), (2, 1)]), (1, "B", [(0, 2), (1, 2)]),
    (1, "B", [(2, 2), (0, 3)]), (1, "B", [(1, 3), (2, 3)]),
    (2, "A", [(0, 0), (0, 1)]), (2, "A", [(1, 0), (1, 1)]),
    (2, "A", [(2, 0), (2, 1)]), (2, "A", [(3, 0), (3, 1)]),
    (3, "B", [(0, 0), (1, 0)]), (3, "B", [(0, 1), (1, 1)]),
    (3, "B", [(0, 2), (1, 2)]), (3, "B", [(0, 3), (1, 3)]),
    (4, "A", [(0, 0), (1, 0)]), (4, "A", [(2, 0), (3, 0)]),
    (5, "B", [(0, 0), (0, 1)]), (5, "B", [(0, 2), (0, 3)]),
    (6, "A", [(0, 0), (0, 1)]), (6, "B", [(0, 2), (1, 2)]),
    (6, "B", [(1, 0), (2, 0)]), (6, "A", [(2, 1), (2, 2)]),
    (6, "A", [(1, 1), None]),
]
assert len(_L1_PLAN) == M1


def _pos(lay, h, w):
    return h * 4 + w if lay == "A" else w * 4 + h


def _chunk_support(ci, grp):
    """Set of (h, w) input positions this chunk's outputs read."""
    kh, kw = KERNELS[ci]
    sup = set()
    for o in grp:
        if o is None:
            continue
        pi, pj = o
        for dh in range(kh):
            for dw in range(kw):
                sup.add((pi + dh, pj + dw))
    return sup


def _build_plan():
    """Resolve each chunk to (layout, half, rows) where rows maps each support
    position to a row block.  Chunks that fit a single 128-row half of layout
    A or B use it; the rest go to the gathered extra layout E (single half).
    Returns the per-chunk source list and the E layout's position list.
    """
    srcs = []
    e_positions = []  # position order within layout E
    for ci, lay_hint, grp in _L1_PLAN:
        sup = _chunk_support(ci, grp)
        placed = None
        for lay in ("A", "B"):
            halves = {_pos(lay, h, w) // 8 for h, w in sup}
            if len(halves) == 1:
                placed = (lay, halves.pop())
                break
        if placed is None:
            # gathered layout: append this chunk's support positions
            for p in sorted(sup):
                if p not in e_positions:
                    e_positions.append(p)
            placed = ("E", 0)
        srcs.append(placed)
    assert len(e_positions) <= 8, e_positions
    return srcs, e_positions


_SRCS, _E_POSITIONS = _build_plan()
N_XE = 1 if _E_POSITIONS else 0


def _row_of(lay, h, w):
    """Row (0..127) of position (h,w), channel 0, within its half of layout."""
    if lay == "E":
        return _E_POSITIONS.index((h, w)) * 16
    p = _pos(lay, h, w)
    return (p - (p // 8) * 8) * 16


def _build_l1_weights(ws, bs):
    """Wc [M1,128,128] bf16 (K-row, M-col), bias bc [M1,128] f32, perm."""
    Wc = np.zeros((M1, P, P), np.float32)
    bch = np.zeros((M1, P), np.float32)
    perm = np.full(H1, -1, np.int64)
    for j, ((ci, _, grp), (lay, half)) in enumerate(zip(_L1_PLAN, _SRCS)):
        kh, kw = KERNELS[ci]
        oh, ow = 5 - kh, 5 - kw
        for sl, o in enumerate(grp):
            if o is None:
                continue
            pi, pj = o
            cols = slice(sl * 64, sl * 64 + 64)
            bch[j, cols] = bs[ci]
            perm[j * P + sl * 64 : j * P + sl * 64 + 64] = (
                OFFSETS[ci] + np.arange(64) * oh * ow + pi * ow + pj
            )
            for dh in range(kh):
                for dw in range(kw):
                    row = _row_of(lay, pi + dh, pj + dw)
                    Wc[j, row : row + 16, cols] = ws[ci][:, :, dh, dw].T
    return Wc, bch, perm


def _x_row_perm():
    """Per layout: new row r -> original flat-x feature index (c*16+h*4+w)."""
    out = {}
    for lay in ("A", "B"):
        a = np.empty(F_IN, np.int64)
        for r in range(F_IN):
            pos, c = r // 16, r % 16
            if lay == "A":
                h, w = pos // 4, pos % 4
            else:
                w, h = pos // 4, pos % 4
            a[r] = c * 16 + h * 4 + w
        out[lay] = a
    e = np.zeros(P, np.int64)
    for i, (h, w) in enumerate(_E_POSITIONS):
        for c in range(16):
            e[i * 16 + c] = c * 16 + h * 4 + w
    out["E"] = e
    return out


_X_PERMS = _x_row_perm()

_PROGRAM_CACHE = {}


# weight-DMA schedule: (tensor, lo, hi) in consumption-need order — small
# fw0 groups first (chunk j's L2 runs at ~3 + 1.07*j us), larger batches
# later to amortize the ~625ns HWDGE fixed cost per DMA (the HWDGE
# descriptor generator is a single shared unit).
_W_SCHED = [
    ("wc", 0, 1), ("cst", 0, 0), ("fw0", 0, 1), ("wc", 1, 4), ("fw0", 1, 2),
    ("fw0", 2, 3), ("wc", 4, 8), ("fw0", 3, 5), ("wc", 8, 16), ("fw0", 5, 8),
    ("wc", 16, M1), ("fw0", 8, 11), ("fw0", 11, 15), ("fw0", 15, 20),
    ("fw0", 20, M1), ("fw1", 0, 0), ("fw2", 0, 0),
]
# All weight DMAs run once on the sync queue (outside the repeat loop):
# pass 1 absorbs a few us of weight-arrival stalls, steady-state passes
# have no weight traffic at all.
NCONST = M1 + M2 + 2  # bc | fb0 | fb1 | fb2


def _build_program(repeat=1, unroll=1):
    nc = bacc.Bacc(None, target_bir_lowering=False)
    # x layouts are partition-major so a whole tile is one DMA.  All matmul
    # operands are bf16 (fp32 PSUM accumulation): same 1 col/cycle PE stream
    # rate as f32r, but the weight-load path gets FWL (disabled for fp32
    # stationary operands), halving LDWEIGHTS and letting it hide under the
    # 512-col streams; x DMA traffic and drain bandwidth also halve.
    xa_d = nc.declare_dram_parameter("xa", [P, K1, B_LOC], BF16, isOutput=False)
    xb_d = nc.declare_dram_parameter("xb", [P, K1, B_LOC], BF16, isOutput=False)
    xe_d = nc.declare_dram_parameter("xe", [P, B_LOC], BF16, isOutput=False)
    wc_d = nc.declare_dram_parameter("wc", [P, M1, P], BF16, isOutput=False)
    fw0_d = nc.declare_dram_parameter("fw0t", [P, M1, H2], BF16, isOutput=False)
    fw1_d = nc.declare_dram_parameter("fw1t", [P, M2, H3], BF16, isOutput=False)
    fw2_d = nc.declare_dram_parameter("fw2t", [P, NA], BF16, isOutput=False)
    cst_d = nc.declare_dram_parameter("cst", [P, NCONST], F32, isOutput=False)
    out_d = nc.declare_dram_parameter("out", [NA, B_LOC], F32, isOutput=True)

    RELU = mybir.ActivationFunctionType.Relu
    ADD = mybir.AluOpType.add
    MAX = mybir.AluOpType.max

    with tile.TileContext(nc) as tc:
        with (
            tc.tile_pool(name="wpool", bufs=1) as wpool,
            tc.tile_pool(name="xpool", bufs=2) as xpool,
            tc.tile_pool(name="a1pool", bufs=1) as a1pool,
            tc.tile_pool(name="a2pool", bufs=2) as a2pool,
            tc.tile_pool(name="a3pool", bufs=2) as a3pool,
            tc.tile_pool(name="opool", bufs=2) as opool,
            tc.tile_pool(name="ps1pool", bufs=3, space="PSUM") as ps1pool,
            tc.tile_pool(name="ps2pool", bufs=1, space="PSUM") as ps2pool,
            tc.tile_pool(name="pstail", bufs=1, space="PSUM") as pstail,
        ):
            # replicated weights, loaded once, resident for the whole pass,
            # in grouped DMAs (deps are per-slice, so chunk j's compute waits
            # only for the group containing j).  Tile 0's layout-A x block
            # goes first on the same queue: it gates the very first matmul
            # and the HWDGE path is ~1us faster than SWDGE.
            wc_all = wpool.tile([P, M1, P], BF16)
            fw0_all = wpool.tile([P, M1, H2], BF16)
            cst = wpool.tile([P, NCONST], F32)
            fw1 = wpool.tile([P, M2, H3], BF16)
            fw2 = wpool.tile([P, NA], BF16)
            for which, g0, g1 in _W_SCHED:
                if which == "wc":
                    nc.sync.dma_start(wc_all[:, g0:g1, :], wc_d[:, g0:g1, :])
                elif which == "fw0":
                    nc.sync.dma_start(fw0_all[:, g0:g1, :], fw0_d[:, g0:g1, :])
                elif which == "cst":
                    nc.sync.dma_start(cst[:], cst_d[:])
                elif which == "fw1":
                    nc.sync.dma_start(fw1[:], fw1_d[:])
                else:
                    nc.sync.dma_start(fw2[:], fw2_d[:])
            bc = cst[:, 0:M1]
            fb0 = cst[:, M1 : M1 + M2]
            fb1 = cst[:, M1 + M2 : M1 + M2 + 1]
            fb2 = cst[0:NA, M1 + M2 + 1 : M1 + M2 + 2]

            def tail_a(tp, a2p):
                # L3 + a3 drain for tile tp; emitted at the NEXT tile
                # boundary so its inputs are long drained and its matmuls
                # cover the a2-accumulator WAR round trip.
                ps3 = pstail.tile([P, NB], F32, tag="pst", name="ps3")
                for k in range(M2):
                    nc.tensor.matmul(
                        ps3[:], fw1[:, k, :], a2p[:, k, :],
                        start=(k == 0), stop=(k == M2 - 1),
                    )
                a3 = a3pool.tile([P, NB], BF16, tag="a3")
                nc.scalar.activation(a3[:], ps3[:], RELU, bias=fb1)
                return a3

            def tail_b(tp, a3):
                # L4 + output, emitted one chunk-group after tail_a so the
                # a3 drain's round trip is covered by ~16 L2 matmuls
                ps4 = pstail.tile([NA, NB], F32, tag="pst", name="ps4")
                nc.tensor.matmul(ps4[:], fw2[:], a3[:], start=True, stop=True)
                ob = opool.tile([NA, NB], F32, tag="ob")
                nc.vector.tensor_scalar_add(ob[:], ps4[:], fb2)
                nc.sync.dma_start(out_d[:, ts(tp, NB)], ob[:])

            def body():
                NCH = BT * M1  # flat chunk stream across all batch tiles
                xts = {}  # tile t -> (xta, xtb, xte)
                a1s = {}
                a2s = {}
                ps2s = {}

                def stage_tile(t):
                    # x DMAs + per-tile SBUF/PSUM allocations for tile t
                    if t >= BT:
                        return
                    xta = xpool.tile([P, K1, NB], BF16, tag="xta", name="xta")
                    if t == 0:
                        for k in range(K1):
                            nc.gpsimd.dma_start(
                                xta[:, k, :], xa_d[:, k, ts(t, NB)]
                            )
                    else:
                        nc.gpsimd.dma_start(xta[:], xa_d[:, :, ts(t, NB)])
                    xtb = xpool.tile([P, K1, NB], BF16, tag="xtb", name="xtb")
                    nc.gpsimd.dma_start(xtb[:], xb_d[:, :, ts(t, NB)])
                    xte = None
                    if N_XE:
                        xte = xpool.tile([P, NB], BF16, tag="xte", name="xte")
                        nc.gpsimd.dma_start(xte[:], xe_d[:, ts(t, NB)])
                    xts[t] = (xta, xtb, xte)
                    a1s[t] = a1pool.tile([P, M1, NB], BF16, tag="a1", name="a1")

                def l1(g):
                    # L1 chunk g: one matmul + relu/bias drain into a1
                    if g >= NCH:
                        return
                    t, j = divmod(g, M1)
                    lay, half = _SRCS[j]
                    xta, xtb, xte = xts[t]
                    if lay == "A":
                        xt = xta[:, half, :]
                    elif lay == "B":
                        xt = xtb[:, half, :]
                    else:
                        xt = xte[:]
                    ps = ps1pool.tile([P, NB], F32, tag="ps1", name="ps1")
                    nc.tensor.matmul(ps[:], wc_all[:, j, :], xt, start=True,
                                     stop=True)
                    a1 = a1s[t]
                    if g % 2 == 0:
                        nc.scalar.activation(
                            a1[:, j, :], ps[:], RELU, bias=bc[:, j : j + 1]
                        )
                    else:
                        nc.vector.tensor_scalar(
                            a1[:, j, :], ps[:], bc[:, j : j + 1], 0.0,
                            ADD, MAX,
                        )

                # chunk groups: L2 is emitted as same-accumulator RUNS of
                # up to G chunks (PSUM-target switches between consecutive
                # matmuls cost real overlap on HW), with the next group's
                # L1 matmuls interleaved one per run.
                G = 4
                GROUPS = [(s, min(s + G, M1)) for s in range(0, M1, G)]
                NGRP = len(GROUPS)

                def group_start(gi):
                    # global chunk index of group gi's first chunk
                    if gi >= BT * NGRP:
                        return NCH
                    t2, gl2 = divmod(gi, NGRP)
                    return t2 * M1 + GROUPS[gl2][0]

                stage_tile(0)
                stage_tile(1)
                for g in range(GROUPS[0][1]):
                    l1(g)
                next_l1 = GROUPS[0][1]
                for gi in range(BT * NGRP):
                    t, gl = divmod(gi, NGRP)
                    j0, j1 = GROUPS[gl]
                    if gl == 0:
                        if t > 0:
                            pending = (t - 1, tail_a(t - 1, a2s.pop(t - 1)))
                        # L2 accumulators allocated at first use so the ring
                        # WAR dep lands on the previous tile's a2 drains
                        ps2s[t] = [
                            ps2pool.tile([P, NB], F32, tag=f"ps2_{m}",
                                         name=f"ps2_{m}")
                            for m in range(M2)
                        ]
                    if gl == 1 and t > 0:
                        tail_b(*pending)
                    a1 = a1s[t]
                    ps2 = ps2s[t]
                    final_group = j1 == M1

                    def drain_a2(a2, m):
                        if m % 2 == 0:
                            nc.vector.tensor_scalar(
                                a2[:, m, :], ps2[m][:],
                                fb0[:, m : m + 1], 0.0, ADD, MAX,
                            )
                        else:
                            nc.scalar.activation(
                                a2[:, m, :], ps2[m][:], RELU,
                                bias=fb0[:, m : m + 1],
                            )

                    if final_group:
                        a2 = a2s[t] = a2pool.tile(
                            [P, M2, NB], BF16, tag="a2", name="a2"
                        )
                    l1_stop = group_start(gi + 2)
                    for m in range(M2):
                        for j in range(j0, j1):
                            nc.tensor.matmul(
                                ps2[m][:],
                                fw0_all[:, j, ts(m, P)],
                                a1[:, j, :],
                                start=(j == 0),
                                stop=(j == M1 - 1),
                            )
                        if final_group:
                            # drain right after the stop matmul: frees the
                            # accumulator early and lets the tail's L3 chase
                            drain_a2(a2, m)
                        # next group's L1 chunks, one per run
                        if next_l1 < l1_stop:
                            l1(next_l1)
                            next_l1 += 1
                    while next_l1 < l1_stop:
                        l1(next_l1)
                        next_l1 += 1
                    if final_group:
                        del ps2s[t]
                        stage_tile(t + 2)
                tail_b(BT - 1, tail_a(BT - 1, a2s.pop(BT - 1)))

            if repeat == 1:
                for _ in range(unroll):
                    body()
            else:
                with tc.For_i(0, repeat, 1):
                    body()

    nc.finalize()
    return nc


def pack_inputs(x, ws, bs, fw0, fb0, fw1, fb1, fw2, fb2):
    """Pack full-problem numpy inputs into the per-core DRAM in_maps."""
    x = np.asarray(x, np.float32).reshape(B, F_IN)
    ws = [np.asarray(w, np.float32) for w in ws]
    bs = [np.asarray(b, np.float32) for b in bs]
    fw0 = np.asarray(fw0, np.float32)
    fb0 = np.asarray(fb0, np.float32)
    fw1 = np.asarray(fw1, np.float32)
    fb1 = np.asarray(fb1, np.float32)
    fw2 = np.asarray(fw2, np.float32)
    fb2 = np.asarray(fb2, np.float32)

    Wc, bch, perm = _build_l1_weights(ws, bs)

    fw0_perm = np.zeros((H2, H1), np.float32)
    valid = perm >= 0
    fw0_perm[:, valid] = fw0[:, perm[valid]]

    # consts [P, NCONST]: bc | fb0 | fb1 | fb2 (fb2 in rows 0..NA-1)
    cst = np.zeros((P, NCONST), np.float32)
    cst[:, :M1] = bch.T
    cst[:, M1 : M1 + M2] = fb0.reshape(M2, P).T
    cst[:, M1 + M2] = fb1
    cst[:NA, M1 + M2 + 1] = fb2

    shared = {
        # partition-major weight layouts: [P(K), chunk, cols], bf16
        "wc": np.ascontiguousarray(Wc.transpose(1, 0, 2)).astype(BF_NP),
        "fw0t": np.ascontiguousarray(
            fw0_perm.T.reshape(M1, P, H2).transpose(1, 0, 2)
        ).astype(BF_NP),
        "fw1t": np.ascontiguousarray(
            fw1.T.reshape(M2, P, H3).transpose(1, 0, 2)
        ).astype(BF_NP),
        "fw2t": np.ascontiguousarray(fw2.T).astype(BF_NP),
        "cst": cst,
    }
    xbf = x.astype(BF_NP)
    in_maps = []
    for i in range(N_CORES):
        shard_t = xbf[i * B_LOC : (i + 1) * B_LOC].T  # [256, B_LOC] view
        xa = np.ascontiguousarray(
            shard_t[_X_PERMS["A"]].reshape(K1, P, B_LOC).transpose(1, 0, 2)
        )
        xb = np.ascontiguousarray(
            shard_t[_X_PERMS["B"]].reshape(K1, P, B_LOC).transpose(1, 0, 2)
        )
        xe = np.ascontiguousarray(shard_t[_X_PERMS["E"]])
        in_maps.append({"xa": xa, "xb": xb, "xe": xe, **shared})
    return in_maps


def kernel(x, w0, b0, w1, b1, w2, b2, w3, b3, w4, b4, w5, b5, w6, b6,
           fw0, fb0, fw1, fb1, fw2, fb2):
    in_maps = pack_inputs(
        x, (w0, w1, w2, w3, w4, w5, w6), (b0, b1, b2, b3, b4, b5, b6),
        fw0, fb0, fw1, fb1, fw2, fb2,
    )
    if "nc" not in _PROGRAM_CACHE:
        _PROGRAM_CACHE["nc"] = _build_program()
    nc = _PROGRAM_CACHE["nc"]

    res = run_bass_kernel_spmd(nc, in_maps, list(range(N_CORES)))
    out = np.concatenate([r["out"] for r in res.results], axis=1)  # [4, B]
    return np.ascontiguousarray(out.T)

